# revision 20
# baseline (speedup 1.0000x reference)
"""AgentAttention Trainium2 kernel — 8-core batch-parallel (2 batches/core).

Decomposition (validated in mirror.py against the reference):
  - host: x transposed to ch-major; q_w pre-scaled by dh^-0.5; k_b dropped
    (softmax-shift-invariant); v_b / dwc_b / proj_b folded into a host-side
    additive correction (softmax rows sum to 1); position biases precomputed
    as exp(bias) factors (multiplicative after exp, avoids PSUM bias adds).
  - device (per batch): Q^T projection ch-major; agent pooling via two
    strided DVE reduces; fused chunk-outer stage 1: K^T/V projected
    just-in-time per 128-token chunk, s1^T in n-partition layout with
    ones-augmented V for the softmax denominator; stage 2 in s2^T
    (agent-partition) layout with block-diagonal head-pair operands;
    denominators via ones-matmul; normalization as per-chunk broadcast
    multiply; depthwise 3x3 conv as 9 fused scalar_tensor_tensor taps on
    ch-major V (edge-cropped regions), split across Vector/GpSimd; final
    projection back to token-major.
"""
import numpy as np
import ml_dtypes

BF = ml_dtypes.bfloat16
NCORES = 8
B = 2              # batches per core
N = 3136
H = W = 56
CT = 4             # 128-channel tiles
HP = 4             # head pairs
A = 49
C7 = 448           # 8 image rows
CH = [(i * 128, min(128, N - i * 128)) for i in range(25)]

_CACHE = {}
DEBUG_TAPS = False


def _lin_weights(in_size, out_size):
    scale = in_size / out_size
    src = (np.arange(out_size, dtype=np.float32) + 0.5) * scale - 0.5
    src = np.maximum(src, 0.0)
    i0 = np.minimum(np.floor(src).astype(np.int32), in_size - 1)
    i1 = np.minimum(i0 + 1, in_size - 1)
    w = (src - i0.astype(np.float32)).astype(np.float32)
    return i0, i1, w


def _resize_matrix(in_size, out_size):
    i0, i1, w = _lin_weights(in_size, out_size)
    M = np.zeros((out_size, in_size), np.float32)
    M[np.arange(out_size), i0] += 1.0 - w
    M[np.arange(out_size), i1] += w
    return M


def _build_nc():
    from contextlib import ExitStack
    import concourse.bass as bass
    import concourse.bacc as bacc
    import concourse.tile as tile
    from concourse import mybir

    fp32 = mybir.dt.float32
    bf16 = mybir.dt.bfloat16
    AF = mybir.ActivationFunctionType
    OP = mybir.AluOpType
    AX = mybir.AxisListType

    nc = bacc.Bacc("TRN2", target_bir_lowering=False)
    xT_d = nc.dram_tensor("xT", (128, B, CT, N), bf16, kind="ExternalInput")
    wqk_d = nc.dram_tensor("wqk", (128, CT, 1024), bf16, kind="ExternalInput")
    wv_d = nc.dram_tensor("wv", (128, CT, 512), bf16, kind="ExternalInput")
    pw_d = nc.dram_tensor("pw", (128, CT, 512), bf16, kind="ExternalInput")
    qsb_d = nc.dram_tensor("qsb", (128, CT), fp32, kind="ExternalInput")
    dwcw_d = nc.dram_tensor("dwcw", (128, 36), fp32, kind="ExternalInput")
    eb1_d = nc.dram_tensor("eb1", (128, 25, HP, 128), bf16, kind="ExternalInput")
    eb2_d = nc.dram_tensor("eb2", (128, HP, 7, C7), bf16, kind="ExternalInput")
    ones_d = nc.dram_tensor("onesbd", (128, 2), bf16, kind="ExternalInput")
    out_d = nc.dram_tensor("out", (B, N, 512), fp32, kind="ExternalOutput")
    rsc_d = nc.dram_tensor("rscratch", (B, 2, HP, N), bf16, kind="Internal")
    if DEBUG_TAPS:
        dbg_qT = nc.dram_tensor("dbg_qT", (128, CT, N), bf16, kind="ExternalOutput")
        dbg_asT = nc.dram_tensor("dbg_asT", (128, CT, A), fp32, kind="ExternalOutput")
        dbg_kch = nc.dram_tensor("dbg_kch", (128, CT, 128), bf16, kind="ExternalOutput")
        dbg_v65 = nc.dram_tensor("dbg_v65", (128, 8, 65), bf16, kind="ExternalOutput")
        dbg_e1 = nc.dram_tensor("dbg_e1", (128, HP, 128), bf16, kind="ExternalOutput")
        dbg_avbd = nc.dram_tensor("dbg_avbd", (128, HP, 128), bf16, kind="ExternalOutput")
        dbg_u = nc.dram_tensor("dbg_u", (128, CT, N), bf16, kind="ExternalOutput")
        dbg_den = nc.dram_tensor("dbg_den", (112, 224), fp32, kind="ExternalOutput")
        dbg_c = nc.dram_tensor("dbg_c", (128, CT, N), bf16, kind="ExternalOutput")

    with ExitStack() as ctx:
        tc = ctx.enter_context(tile.TileContext(nc))
        consts = ctx.enter_context(tc.tile_pool(name="consts", bufs=1))
        big = ctx.enter_context(tc.tile_pool(name="big", bufs=1))
        xu = ctx.enter_context(tc.tile_pool(name="xu", bufs=2))
        qp = ctx.enter_context(tc.tile_pool(name="qp", bufs=2))
        biasp = ctx.enter_context(tc.tile_pool(name="biasp", bufs=2))
        work = ctx.enter_context(tc.tile_pool(name="work", bufs=3))
        batch1 = ctx.enter_context(tc.tile_pool(name="batch1", bufs=1))
        perb = ctx.enter_context(tc.tile_pool(name="perb", bufs=2))
        ps_mm = ctx.enter_context(tc.tile_pool(name="psmm", bufs=3, space="PSUM"))
        ps_av = ctx.enter_context(tc.tile_pool(name="psav", bufs=4, space="PSUM"))
        ps_sm = ctx.enter_context(tc.tile_pool(name="pssm", bufs=1, space="PSUM"))

        wqk_s = consts.tile([128, CT, 1024], bf16)
        nc.sync.dma_start(out=wqk_s, in_=wqk_d[:, :, :])
        wv_s = consts.tile([128, CT, 512], bf16)
        nc.sync.dma_start(out=wv_s, in_=wv_d[:, :, :])
        pw_s = consts.tile([128, CT, 512], bf16)
        nc.sync.dma_start(out=pw_s, in_=pw_d[:, :, :])
        qsb_s = consts.tile([128, CT], fp32)
        nc.sync.dma_start(out=qsb_s, in_=qsb_d[:, :])
        dwcw_s = consts.tile([128, 36], fp32)
        nc.sync.dma_start(out=dwcw_s, in_=dwcw_d[:, :])
        onesbd = consts.tile([128, 2], bf16)
        nc.sync.dma_start(out=onesbd, in_=ones_d[:, :])

        for b in range(B):
            xT = xu.tile([128, CT, N], bf16, tag="xu")
            nc.sync.dma_start(out=xT, in_=xT_d[:, b, :, :])
            qT = qp.tile([128, CT, N], bf16, tag="qT")

            # ---- Q^T (ch-major, + bias, pre-scaled) ----
            for ct in range(CT):
                for c in range(7):
                    ps = ps_mm.tile([128, 512], fp32, tag="mm")
                    for kt in range(CT):
                        nc.tensor.matmul(
                            ps[:, 0:C7],
                            wqk_s[:, kt, ct * 128:(ct + 1) * 128],
                            xT[:, kt, c * C7:(c + 1) * C7],
                            start=(kt == 0), stop=(kt == 3),
                        )
                    nc.scalar.activation(
                        out=qT[:, ct, c * C7:(c + 1) * C7], in_=ps[:, 0:C7],
                        func=AF.Identity, bias=qsb_s[:, ct:ct + 1], scale=1.0,
                    )

            # ---- agent pooling -> block-diag head-pair operands ----
            asT = batch1.tile([128, CT, A], fp32, tag="asT")
            bd1 = batch1.tile([128, CT, 128], bf16, tag="bd1")
            bd2 = batch1.tile([128, CT, 128], bf16, tag="bd2")
            nc.vector.memset(bd1, 0.0)
            nc.vector.memset(bd2, 0.0)
            for ct in range(CT):
                p1 = work.tile([128, 392], fp32, tag="pool1")
                nc.vector.reduce_sum(
                    out=p1.rearrange("p (y q) -> p y q", y=56),
                    in_=qT[:, ct, :].rearrange("p (y q r) -> p y q r", y=56, q=7),
                    axis=AX.X,
                )
                nc.vector.reduce_sum(
                    out=asT[:, ct, :].rearrange("p (a c) -> p a c", a=7),
                    in_=p1.rearrange("p (yq yr xq) -> p yq xq yr", yq=7, yr=8),
                    axis=AX.X,
                )
                nc.scalar.mul(out=bd1[0:64, ct, 0:49], in_=asT[0:64, ct, :], mul=1.0 / 64.0)
                nc.scalar.mul(out=bd1[64:128, ct, 64:113], in_=asT[64:128, ct, :], mul=1.0 / 64.0)
                nc.scalar.mul(out=bd2[0:64, ct, 0:49], in_=asT[0:64, ct, :], mul=1.0 / 8.0)
                nc.scalar.mul(out=bd2[64:128, ct, 64:113], in_=asT[64:128, ct, :], mul=1.0 / 8.0)

            if DEBUG_TAPS and b == 0:
                nc.sync.dma_start(out=dbg_qT[:, :, :], in_=qT)
                nc.sync.dma_start(out=dbg_asT[:, :, :], in_=asT)
            # ---- fused stage 1 (chunk-outer): K/V projected just-in-time ----
            avps = []
            for _hp in range(HP):
                avp = ps_av.tile([128, 130], fp32, tag="av")
                avps.append(avp)
            for ci, (t0, cs) in enumerate(CH):
                # V chunk (token-major, 65-stride head blocks + ones col)
                v65 = perb.tile([128, 8, 65], bf16, tag="v65")
                ps = ps_mm.tile([128, 512], fp32, tag="mm")
                for kt in range(CT):
                    nc.tensor.matmul(
                        ps[0:cs, :], xT[:, kt, t0:t0 + cs], wv_s[:, kt, :],
                        start=(kt == 0), stop=(kt == 3),
                    )
                nc.vector.tensor_copy(
                    out=v65[0:cs, :, 0:64],
                    in_=ps[0:cs, :].rearrange("p (h d) -> p h d", h=8),
                )
                nc.vector.memset(v65[0:cs, :, 64:65], 1.0)
                # K^T chunk (ch-major)
                kch = perb.tile([128, CT, 128], bf16, tag="kch")
                for ct in range(CT):
                    psk = ps_mm.tile([128, 512], fp32, tag="mm")
                    for kt in range(CT):
                        nc.tensor.matmul(
                            psk[:, 0:cs],
                            wqk_s[:, kt, 512 + ct * 128:512 + (ct + 1) * 128],
                            xT[:, kt, t0:t0 + cs],
                            start=(kt == 0), stop=(kt == 3),
                        )
                    nc.scalar.copy(out=kch[:, ct, 0:cs], in_=psk[:, 0:cs])
                if DEBUG_TAPS and b == 0 and ci == 0:
                    nc.sync.dma_start(out=dbg_kch[:, :, :], in_=kch)
                    nc.sync.dma_start(out=dbg_v65[:, :, :], in_=v65)
                # biases for this chunk (all head pairs)
                e1b = biasp.tile([128, HP, 128], bf16, tag="eb1")
                nc.sync.dma_start(out=e1b, in_=eb1_d[:, ci, :, :])
                for hp in range(HP):
                    ps1 = ps_mm.tile([128, 512], fp32, tag="mm")
                    nc.tensor.matmul(
                        ps1[0:cs, 0:128], kch[:, hp, 0:cs], bd1[:, hp, :],
                        start=True, stop=True,
                    )
                    et = work.tile([128, 128], bf16, tag="e1")
                    nc.scalar.activation(out=et[0:cs, :], in_=ps1[0:cs, 0:128], func=AF.Exp)
                    nc.vector.tensor_mul(out=et[0:cs, :], in0=et[0:cs, :], in1=e1b[0:cs, hp, :])
                    if DEBUG_TAPS and b == 0 and ci == 0:
                        nc.sync.dma_start(out=dbg_e1[:, hp, :], in_=et[:, :])
                    nc.tensor.matmul(
                        avps[hp][:, :],
                        et[0:cs, :],
                        v65[0:cs, 2 * hp:2 * hp + 2, :],
                        start=(ci == 0), stop=(ci == 24),
                    )
            # normalize agent_v into block-diag lhsT tiles
            avbds = []
            for hp in range(HP):
                avbd = batch1.tile([128, 128], bf16, tag=f"avbd{hp}")
                nc.vector.memset(avbd, 0.0)
                rr = work.tile([128, 1], fp32, tag="rr")
                for e in range(2):
                    nc.vector.reciprocal(out=rr[64 * e:64 * e + 49, :],
                                         in_=avps[hp][64 * e:64 * e + 49, 65 * e + 64:65 * e + 65])
                    nc.vector.tensor_scalar_mul(
                        out=avbd[64 * e:64 * e + 49, 64 * e:64 * e + 64],
                        in0=avps[hp][64 * e:64 * e + 49, 65 * e:65 * e + 64],
                        scalar1=rr[64 * e:64 * e + 49, :],
                    )
                if DEBUG_TAPS and b == 0:
                    nc.sync.dma_start(out=dbg_avbd[:, hp, :], in_=avbd)
                avbds.append(avbd)

            # ---- V^T (ch-major) for the depthwise conv ----
            vT = big.tile([128, CT, N], bf16, tag="vT")
            for ct in range(CT):
                for r in range(7):
                    ps = ps_mm.tile([128, 512], fp32, tag="mm")
                    for kt in range(CT):
                        nc.tensor.matmul(
                            ps[:, 0:C7],
                            wv_s[:, kt, ct * 128:(ct + 1) * 128],
                            xT[:, kt, r * C7:(r + 1) * C7],
                            start=(kt == 0), stop=(kt == 3),
                        )
                    nc.scalar.copy(out=vT[:, ct, r * C7:(r + 1) * C7], in_=ps[:, 0:C7])

            # ---- stage 2 per head pair: s2^T, E2, U + densum ----
            u_s = xu.tile([128, CT, N], bf16, tag="xu")
            denpk = batch1.tile([112, 224], fp32, tag="denpk")
            for hp in range(HP):
                for c in range(7):
                    e2b = biasp.tile([128, C7], bf16, tag="eb2")
                    nc.sync.dma_start(out=e2b, in_=eb2_d[:, hp, c, :])
                    ps2 = ps_mm.tile([128, 512], fp32, tag="mm")
                    nc.tensor.matmul(
                        ps2[0:128, 0:C7], bd2[:, hp, :], qT[:, hp, c * C7:(c + 1) * C7],
                        start=True, stop=True,
                    )
                    et2 = work.tile([128, C7], bf16, tag="e2")
                    nc.scalar.activation(out=et2, in_=ps2[0:128, 0:C7], func=AF.Exp)
                    nc.vector.tensor_mul(out=et2, in0=et2, in1=e2b)
                    psU = ps_mm.tile([128, 512], fp32, tag="mm")
                    nc.tensor.matmul(psU[:, 0:C7], avbds[hp], et2, start=True, stop=True)
                    psD = ps_sm.tile([2, C7], fp32, tag="sm")
                    nc.tensor.matmul(psD, onesbd, et2, start=True, stop=True)
                    nc.scalar.copy(out=u_s[:, hp, c * C7:(c + 1) * C7], in_=psU[:, 0:C7])
                    dtmp = work.tile([2, C7], fp32, tag="dtmp")
                    nc.scalar.copy(out=dtmp, in_=psD)
                    for e in range(2):
                        nc.sync.dma_start(
                            out=denpk[e * 56 + hp * 14 + 2 * c:e * 56 + hp * 14 + 2 * c + 2, :],
                            in_=dtmp[e:e + 1, :])

            # ---- normalize U, add depthwise conv, project ----
            if DEBUG_TAPS and b == 0:
                nc.sync.dma_start(out=dbg_u[:, :, :], in_=u_s)
                nc.sync.dma_start(out=dbg_den[:, :], in_=denpk)
            rpk = batch1.tile([112, 224], bf16, tag="rpk")
            with nc.allow_low_precision(reason="single bf16 rounding of 1/den"):
                nc.vector.reciprocal(out=rpk, in_=denpk)
            nc.sync.dma_start(out=rsc_d[b, :, :, :], in_=rpk)
            for ct in range(CT):
                for c in range(7):
                    rbc = work.tile([128, C7], bf16, tag="rbc")
                    nc.sync.dma_start(
                        out=rbc[0:64, :],
                        in_=rsc_d[b, 0:1, ct, c * C7:(c + 1) * C7].to_broadcast((64, C7)))
                    nc.sync.dma_start(
                        out=rbc[64:128, :],
                        in_=rsc_d[b, 1:2, ct, c * C7:(c + 1) * C7].to_broadcast((64, C7)))
                    nc.vector.tensor_mul(
                        out=u_s[:, ct, c * C7:(c + 1) * C7],
                        in0=u_s[:, ct, c * C7:(c + 1) * C7], in1=rbc)
                # depthwise 3x3: edge-cropped fused multiply-add taps
                uv = u_s[:, ct, :].rearrange("p (y x) -> p y x", y=56)
                vv = vT[:, ct, :].rearrange("p (y x) -> p y x", y=56)
                for j in range(9):
                    dy, dx = j // 3 - 1, j % 3 - 1
                    y0 = max(0, -dy); y1 = min(40, 56 - dy)
                    x0 = max(0, -dx); x1 = min(56, 56 - dx)
                    nc.vector.scalar_tensor_tensor(
                        out=uv[:, y0:y1, x0:x1],
                        in0=vv[:, y0 + dy:y1 + dy, x0 + dx:x1 + dx],
                        scalar=dwcw_s[:, ct * 9 + j:ct * 9 + j + 1],
                        in1=uv[:, y0:y1, x0:x1],
                        op0=OP.mult, op1=OP.add,
                    )
                for j in range(9):
                    dy, dx = j // 3 - 1, j % 3 - 1
                    y0 = 40; y1 = min(56, 56 - dy)
                    x0 = max(0, -dx); x1 = min(56, 56 - dx)
                    gtmp = work.tile([128, 16, 56], bf16, tag="gtmp")
                    nc.gpsimd.tensor_scalar_mul(
                        out=gtmp[:, 0:y1 - y0, x0:x1],
                        in0=vv[:, y0 + dy:y1 + dy, x0 + dx:x1 + dx],
                        scalar1=dwcw_s[:, ct * 9 + j:ct * 9 + j + 1],
                    )
                    nc.gpsimd.tensor_add(
                        out=uv[:, y0:y1, x0:x1],
                        in0=uv[:, y0:y1, x0:x1],
                        in1=gtmp[:, 0:y1 - y0, x0:x1],
                    )
            if DEBUG_TAPS and b == 0:
                nc.sync.dma_start(out=dbg_c[:, :, :], in_=u_s)
            for ci, (t0, cs) in enumerate(CH):
                psP = ps_mm.tile([128, 512], fp32, tag="mm")
                for kt in range(CT):
                    nc.tensor.matmul(
                        psP[0:cs, :], u_s[:, kt, t0:t0 + cs], pw_s[:, kt, :],
                        start=(kt == 0), stop=(kt == 3),
                    )
                ot = work.tile([128, 512], fp32, tag="ot")
                nc.scalar.copy(out=ot[0:cs, :], in_=psP[0:cs, :])
                nc.sync.dma_start(out=out_d[b, t0:t0 + cs, :], in_=ot[0:cs, :])
    return nc


def _host_prep(q_w, q_b, kv_w, kv_b, proj_w, proj_b, dwc_w, dwc_b,
               an_bias, na_bias, ah_bias, aw_bias, ha_bias, wa_bias):
    heads, dh = 8, 64
    scale = dh ** -0.5
    q_w = np.asarray(q_w, np.float32); q_b = np.asarray(q_b, np.float32)
    kv_w = np.asarray(kv_w, np.float32); kv_b = np.asarray(kv_b, np.float32)
    proj_w = np.asarray(proj_w, np.float32); proj_b = np.asarray(proj_b, np.float32)
    dwc_w = np.asarray(dwc_w, np.float32); dwc_b = np.asarray(dwc_b, np.float32)

    Rh = _resize_matrix(7, H)
    Rw = _resize_matrix(7, W)
    an = np.asarray(an_bias, np.float32); na = np.asarray(na_bias, np.float32)
    pb1 = np.einsum('yi,haij,xj->hayx', Rh, an, Rw).reshape(heads, A, N)
    pb2 = (np.asarray(ah_bias, np.float32) + np.asarray(aw_bias, np.float32)).reshape(heads, A, N)
    bias1 = pb1 + pb2                                      # (h, a, n)
    ab1 = np.einsum('yi,haij,xj->hayx', Rh, na, Rw).reshape(heads, A, N)
    ab2 = (np.asarray(ha_bias, np.float32) + np.asarray(wa_bias, np.float32)).reshape(heads, N, A)
    bias2 = ab1.transpose(0, 2, 1) + ab2                   # (h, n, a)

    k_w = kv_w[:, :512]
    v_w = kv_w[:, 512:]
    v_b = kv_b[512:]
    dwc9 = dwc_w.reshape(512, 9)

    wqk = np.concatenate([q_w * scale, k_w], axis=1)       # (512, 1024)
    wqk_t = np.ascontiguousarray(wqk.reshape(4, 128, 1024).transpose(1, 0, 2)).astype(BF)
    wv_t = np.ascontiguousarray(v_w.reshape(4, 128, 512).transpose(1, 0, 2)).astype(BF)
    pw_t = np.ascontiguousarray(proj_w.reshape(4, 128, 512).transpose(1, 0, 2)).astype(BF)
    qsb_t = np.ascontiguousarray((q_b * scale).reshape(4, 128).T).astype(np.float32)
    dwcw_t = np.ascontiguousarray(dwc9.reshape(4, 128, 9).transpose(1, 0, 2).reshape(128, 36)).astype(np.float32)

    # eb1 (128, 25, HP, 128): [p, ch, hp, 64e+a] = exp(bias1)[2hp+e, a, 128ch+p]
    e1 = np.exp(bias1)                                     # (h, a, n)
    e1p = np.ones((128, 25, HP, 128), np.float32)
    e1t = e1.transpose(2, 0, 1)                            # (n, h, a)
    for ci, (t0, cs) in enumerate(CH):
        blk = e1t[t0:t0 + cs]                              # (cs, h, a)
        for hp_ in range(HP):
            e1p[:cs, ci, hp_, 0:49] = blk[:, 2 * hp_, :]
            e1p[:cs, ci, hp_, 64:113] = blk[:, 2 * hp_ + 1, :]
    eb1_t = e1p.astype(BF)

    # eb2 (128, HP, 7, 448): [64e+a, hp, c, t'] = exp(bias2)[2hp+e, 448c+t', a]
    e2 = np.exp(bias2)                                     # (h, n, a)
    e2p = np.zeros((128, HP, 7, C7), np.float32)
    for hp_ in range(HP):
        for e in range(2):
            e2p[64 * e:64 * e + 49, hp_] = e2[2 * hp_ + e].reshape(7, C7, A).transpose(2, 0, 1)
    eb2_t = e2p.astype(BF)

    # onesbd (128, 2): col e = 1 on rows 64e..64e+48, else 0
    ones_t = np.zeros((128, 2), np.float32)
    ones_t[0:49, 0] = 1.0
    ones_t[64:113, 1] = 1.0
    ones_t = ones_t.astype(BF)

    # host additive correction (v_b + dwc_b + proj_b, exact via softmax-sum-1)
    Mv = np.zeros((9, H, W), np.float32)
    for j in range(9):
        dy, dx = j // 3 - 1, j % 3 - 1
        Mv[j, max(0, -dy):H - max(0, dy), max(0, -dx):W - max(0, dx)] = 1.0
    S = np.einsum('jt,cj->tc', Mv.reshape(9, N), dwc9)
    corr = v_b[None, :] * (1.0 + S) + dwc_b[None, :]
    corr_out = (corr @ proj_w + proj_b[None, :]).astype(np.float32)   # (n, 512)

    return dict(wqk=wqk_t, wv=wv_t, pw=pw_t, qsb=qsb_t, dwcw=dwcw_t,
                eb1=eb1_t, eb2=eb2_t, onesbd=ones_t), corr_out


def kernel(**inputs):
    from concourse.bass_utils import run_bass_kernel_spmd

    x = np.asarray(inputs['x'], np.float32)                # (16, 3136, 512)
    shared, corr_out = _host_prep(
        inputs['q_w'], inputs['q_b'], inputs['kv_w'], inputs['kv_b'],
        inputs['proj_w'], inputs['proj_b'], inputs['dwc_w'], inputs['dwc_b'],
        inputs['an_bias'], inputs['na_bias'], inputs['ah_bias'],
        inputs['aw_bias'], inputs['ha_bias'], inputs['wa_bias'])

    # xT per core: (128, B, CT, N) bf16 ; [p, b, kt, t] = x[2c+b, t, 128kt+p]
    xb = np.ascontiguousarray(
        x.reshape(NCORES, B, N, CT, 128).transpose(0, 4, 1, 3, 2)).astype(BF)

    if 'nc' not in _CACHE:
        nc = _build_nc()
        nc.finalize()
        _CACHE['nc'] = nc
    nc = _CACHE['nc']

    in_maps = []
    for c in range(NCORES):
        m = {'xT': xb[c]}
        m.update(shared)
        in_maps.append(m)
    res = run_bass_kernel_spmd(nc, in_maps, core_ids=list(range(NCORES)))
    outs = res.results
    full = np.concatenate([np.asarray(o['out']).reshape(B, N, 512) for o in outs], axis=0)
    full = full + corr_out[None, :, :]
    return full.astype(np.float32)


# revision 21
# speedup vs baseline: 2.3846x; 2.3846x over previous
"""AgentAttention Trainium2 kernel — 8-core batch-parallel (2 batches/core).

Decomposition (validated in mirror.py against the reference):
  - host: x transposed to ch-major; q_w pre-scaled by dh^-0.5; k_b dropped
    (softmax-shift-invariant); v_b / dwc_b / proj_b folded into a host-side
    additive correction (softmax rows sum to 1); position biases precomputed
    as exp(bias) factors (multiplicative after exp, avoids PSUM bias adds).
  - device (per batch): Q^T projection ch-major; agent pooling via two
    strided DVE reduces; fused chunk-outer stage 1: K^T/V projected
    just-in-time per 128-token chunk, s1^T in n-partition layout with
    ones-augmented V for the softmax denominator; stage 2 in s2^T
    (agent-partition) layout with block-diagonal head-pair operands;
    denominators via ones-matmul; normalization as per-chunk broadcast
    multiply; depthwise 3x3 conv as 9 fused scalar_tensor_tensor taps on
    ch-major V (edge-cropped regions), split across Vector/GpSimd; final
    projection back to token-major.
"""
import numpy as np
import ml_dtypes

BF = ml_dtypes.bfloat16
NCORES = 8
B = 2              # batches per core
N = 3136
H = W = 56
CT = 4             # 128-channel tiles
HP = 4             # head pairs
A = 49
C7 = 448           # 8 image rows
CH = [(i * 128, min(128, N - i * 128)) for i in range(25)]

_CACHE = {}
DEBUG_TAPS = False


def _lin_weights(in_size, out_size):
    scale = in_size / out_size
    src = (np.arange(out_size, dtype=np.float32) + 0.5) * scale - 0.5
    src = np.maximum(src, 0.0)
    i0 = np.minimum(np.floor(src).astype(np.int32), in_size - 1)
    i1 = np.minimum(i0 + 1, in_size - 1)
    w = (src - i0.astype(np.float32)).astype(np.float32)
    return i0, i1, w


def _resize_matrix(in_size, out_size):
    i0, i1, w = _lin_weights(in_size, out_size)
    M = np.zeros((out_size, in_size), np.float32)
    M[np.arange(out_size), i0] += 1.0 - w
    M[np.arange(out_size), i1] += w
    return M


def _build_nc():
    from contextlib import ExitStack
    import concourse.bacc as bacc
    import concourse.tile as tile
    from concourse import mybir

    fp32 = mybir.dt.float32
    bf16 = mybir.dt.bfloat16
    AF = mybir.ActivationFunctionType
    OP = mybir.AluOpType
    AX = mybir.AxisListType

    nc = bacc.Bacc("TRN2", target_bir_lowering=False)
    xT_d = nc.dram_tensor("xT", (128, B, CT, N), bf16, kind="ExternalInput")
    wqk_d = nc.dram_tensor("wqk", (128, CT, 1024), bf16, kind="ExternalInput")
    wv_d = nc.dram_tensor("wv", (128, CT, 512), bf16, kind="ExternalInput")
    pw_d = nc.dram_tensor("pw", (128, CT, 512), bf16, kind="ExternalInput")
    qsb_d = nc.dram_tensor("qsb", (128, CT), fp32, kind="ExternalInput")
    wdiag_d = nc.dram_tensor("wdiag", (128, 36, 128), bf16, kind="ExternalInput")
    eb1_d = nc.dram_tensor("eb1", (128, 25, HP, 128), bf16, kind="ExternalInput")
    eb2_d = nc.dram_tensor("eb2", (128, HP, 7, C7), bf16, kind="ExternalInput")
    ones_d = nc.dram_tensor("onesbd", (128, 2), bf16, kind="ExternalInput")
    out_d = nc.dram_tensor("out", (B, N, 512), fp32, kind="ExternalOutput")
    rsc_d = nc.dram_tensor("rscratch", (B, 2, HP, N), bf16, kind="Internal")

    with ExitStack() as ctx:
        tc = ctx.enter_context(tile.TileContext(nc))
        consts = ctx.enter_context(tc.tile_pool(name="consts", bufs=1))
        xu = ctx.enter_context(tc.tile_pool(name="xu", bufs=2))
        qp = ctx.enter_context(tc.tile_pool(name="qp", bufs=1))
        kv = ctx.enter_context(tc.tile_pool(name="kv", bufs=2))
        biasp = ctx.enter_context(tc.tile_pool(name="biasp", bufs=2))
        work = ctx.enter_context(tc.tile_pool(name="work", bufs=3))
        batch1 = ctx.enter_context(tc.tile_pool(name="batch1", bufs=1))
        perb = ctx.enter_context(tc.tile_pool(name="perb", bufs=2))
        rbcp = ctx.enter_context(tc.tile_pool(name="rbcp", bufs=2))
        ps_mm = ctx.enter_context(tc.tile_pool(name="psmm", bufs=3, space="PSUM"))
        ps_av = ctx.enter_context(tc.tile_pool(name="psav", bufs=4, space="PSUM"))
        ps_sm = ctx.enter_context(tc.tile_pool(name="pssm", bufs=1, space="PSUM"))

        wqk_s = consts.tile([128, CT, 1024], bf16)
        nc.sync.dma_start(out=wqk_s, in_=wqk_d[:, :, :])
        wv_s = consts.tile([128, CT, 512], bf16)
        nc.sync.dma_start(out=wv_s, in_=wv_d[:, :, :])
        pw_s = consts.tile([128, CT, 512], bf16)
        nc.sync.dma_start(out=pw_s, in_=pw_d[:, :, :])
        qsb_s = consts.tile([128, CT], fp32)
        nc.sync.dma_start(out=qsb_s, in_=qsb_d[:, :])
        wdiag_s = consts.tile([128, 36, 128], bf16)
        nc.sync.dma_start(out=wdiag_s, in_=wdiag_d[:, :, :])
        onesbd = consts.tile([128, 2], bf16)
        nc.sync.dma_start(out=onesbd, in_=ones_d[:, :])

        for b in range(B):
            xT = xu.tile([128, CT, N], bf16, tag="xu")
            nc.sync.dma_start(out=xT, in_=xT_d[:, b, :, :])
            qT = qp.tile([128, CT, N], bf16, tag="qT")

            # ---- Q^T (ch-major, + bias, pre-scaled) ----
            for ct in range(CT):
                for c in range(7):
                    ps = ps_mm.tile([128, 512], fp32, tag="mm")
                    for kt in range(CT):
                        nc.tensor.matmul(
                            ps[:, 0:C7],
                            wqk_s[:, kt, ct * 128:(ct + 1) * 128],
                            xT[:, kt, c * C7:(c + 1) * C7],
                            start=(kt == 0), stop=(kt == 3),
                        )
                    nc.scalar.activation(
                        out=qT[:, ct, c * C7:(c + 1) * C7], in_=ps[:, 0:C7],
                        func=AF.Identity, bias=qsb_s[:, ct:ct + 1], scale=1.0,
                    )

            # ---- K^T (ch-major, resident) ----
            kT = kv.tile([128, CT, N], bf16, tag="kT")
            for ct in range(CT):
                for c in range(7):
                    ps = ps_mm.tile([128, 512], fp32, tag="mm")
                    for kt in range(CT):
                        nc.tensor.matmul(
                            ps[:, 0:C7],
                            wqk_s[:, kt, 512 + ct * 128:512 + (ct + 1) * 128],
                            xT[:, kt, c * C7:(c + 1) * C7],
                            start=(kt == 0), stop=(kt == 3),
                        )
                    nc.scalar.copy(out=kT[:, ct, c * C7:(c + 1) * C7], in_=ps[:, 0:C7])

            # ---- agent pooling -> block-diag head-pair operands ----
            asT = batch1.tile([128, CT, A], fp32, tag="asT")
            bd1 = batch1.tile([128, CT, 128], bf16, tag="bd1")
            bd2 = batch1.tile([128, CT, 128], bf16, tag="bd2")
            nc.vector.memset(bd1, 0.0)
            nc.vector.memset(bd2, 0.0)
            for ct in range(CT):
                p1 = work.tile([128, 392], fp32, tag="pool1")
                nc.vector.reduce_sum(
                    out=p1.rearrange("p (y q) -> p y q", y=56),
                    in_=qT[:, ct, :].rearrange("p (y q r) -> p y q r", y=56, q=7),
                    axis=AX.X,
                )
                nc.vector.reduce_sum(
                    out=asT[:, ct, :].rearrange("p (a c) -> p a c", a=7),
                    in_=p1.rearrange("p (yq yr xq) -> p yq xq yr", yq=7, yr=8),
                    axis=AX.X,
                )
                nc.scalar.mul(out=bd1[0:64, ct, 0:49], in_=asT[0:64, ct, :], mul=1.0 / 64.0)
                nc.scalar.mul(out=bd1[64:128, ct, 64:113], in_=asT[64:128, ct, :], mul=1.0 / 64.0)
                nc.scalar.mul(out=bd2[0:64, ct, 0:49], in_=asT[0:64, ct, :], mul=1.0 / 8.0)
                nc.scalar.mul(out=bd2[64:128, ct, 64:113], in_=asT[64:128, ct, :], mul=1.0 / 8.0)

            # ---- fused stage 1 (chunk-outer): all 4 head pairs per chunk ----
            avps = []
            for _hp in range(HP):
                avp = ps_av.tile([128, 130], fp32, tag="av")
                avps.append(avp)
            for ci, (t0, cs) in enumerate(CH):
                v65 = perb.tile([128, 8, 65], bf16, tag="v65")
                ps = ps_mm.tile([128, 512], fp32, tag="mm")
                for kt in range(CT):
                    nc.tensor.matmul(
                        ps[0:cs, :], xT[:, kt, t0:t0 + cs], wv_s[:, kt, :],
                        start=(kt == 0), stop=(kt == 3),
                    )
                nc.vector.tensor_copy(
                    out=v65[0:cs, :, 0:64],
                    in_=ps[0:cs, :].rearrange("p (h d) -> p h d", h=8),
                )
                nc.vector.memset(v65[0:cs, :, 64:65], 1.0)
                e1b = biasp.tile([128, HP, 128], bf16, tag="eb1")
                nc.sync.dma_start(out=e1b, in_=eb1_d[:, ci, :, :])
                ps1 = ps_mm.tile([128, 512], fp32, tag="mm")
                for hp in range(HP):
                    nc.tensor.matmul(
                        ps1[0:cs, 128 * hp:128 * hp + 128],
                        kT[:, hp, t0:t0 + cs], bd1[:, hp, :],
                        start=True, stop=True,
                    )
                et4 = work.tile([128, HP, 128], bf16, tag="e1")
                nc.scalar.activation(
                    out=et4[0:cs, :, :].rearrange("p h a -> p (h a)"),
                    in_=ps1[0:cs, 0:512], func=AF.Exp)
                nc.vector.tensor_mul(out=et4[0:cs, :, :], in0=et4[0:cs, :, :], in1=e1b[0:cs, :, :])
                for hp in range(HP):
                    nc.tensor.matmul(
                        avps[hp][:, :],
                        et4[0:cs, hp, :],
                        v65[0:cs, 2 * hp:2 * hp + 2, :],
                        start=(ci == 0), stop=(ci == 24),
                    )
            # normalize agent_v into block-diag lhsT tiles
            avbds = []
            for hp in range(HP):
                avbd = batch1.tile([128, 128], bf16, tag=f"avbd{hp}")
                nc.vector.memset(avbd, 0.0)
                rr = work.tile([128, 1], fp32, tag="rr")
                for e in range(2):
                    nc.vector.reciprocal(out=rr[64 * e:64 * e + 49, :],
                                         in_=avps[hp][64 * e:64 * e + 49, 65 * e + 64:65 * e + 65])
                    nc.vector.tensor_scalar_mul(
                        out=avbd[64 * e:64 * e + 49, 64 * e:64 * e + 64],
                        in0=avps[hp][64 * e:64 * e + 49, 65 * e:65 * e + 64],
                        scalar1=rr[64 * e:64 * e + 49, :],
                    )
                avbds.append(avbd)

            # ---- V^T (ch-major, zero-padded image) for the depthwise conv ----
            vpad = kv.tile([128, CT, 58, 58], bf16, tag="kT")
            nc.vector.memset(vpad, 0.0)
            for ct in range(CT):
                for r in range(7):
                    ps = ps_mm.tile([128, 512], fp32, tag="mm")
                    for kt in range(CT):
                        nc.tensor.matmul(
                            ps[:, 0:C7],
                            wv_s[:, kt, ct * 128:(ct + 1) * 128],
                            xT[:, kt, r * C7:(r + 1) * C7],
                            start=(kt == 0), stop=(kt == 3),
                        )
                    nc.scalar.copy(
                        out=vpad[:, ct, 1 + 8 * r:9 + 8 * r, 1:57],
                        in_=ps[:, 0:C7].rearrange("p (y x) -> p y x", y=8))

            # ---- stage 2 per head pair: s2^T, E2, U + densum ----
            u_s = xu.tile([128, CT, N], bf16, tag="xu")
            denpk = batch1.tile([112, 224], fp32, tag="denpk")
            for hp in range(HP):
                for c in range(7):
                    e2b = biasp.tile([128, C7], bf16, tag="eb2")
                    nc.sync.dma_start(out=e2b, in_=eb2_d[:, hp, c, :])
                    ps2 = ps_mm.tile([128, 512], fp32, tag="mm")
                    nc.tensor.matmul(
                        ps2[0:128, 0:C7], bd2[:, hp, :], qT[:, hp, c * C7:(c + 1) * C7],
                        start=True, stop=True,
                    )
                    et2 = work.tile([128, C7], bf16, tag="e2")
                    nc.scalar.activation(out=et2, in_=ps2[0:128, 0:C7], func=AF.Exp)
                    nc.vector.tensor_mul(out=et2, in0=et2, in1=e2b)
                    psU = ps_mm.tile([128, 512], fp32, tag="mm")
                    nc.tensor.matmul(psU[:, 0:C7], avbds[hp], et2, start=True, stop=True)
                    psD = ps_sm.tile([2, C7], fp32, tag="sm")
                    nc.tensor.matmul(psD, onesbd, et2, start=True, stop=True)
                    nc.scalar.copy(out=u_s[:, hp, c * C7:(c + 1) * C7], in_=psU[:, 0:C7])
                    dtmp = work.tile([2, C7], fp32, tag="dtmp")
                    nc.scalar.copy(out=dtmp, in_=psD)
                    for e in range(2):
                        nc.gpsimd.dma_start(
                            out=denpk[e * 56 + hp * 14 + 2 * c:e * 56 + hp * 14 + 2 * c + 2, :],
                            in_=dtmp[e:e + 1, :])
            rpk = batch1.tile([112, 224], bf16, tag="rpk")
            with nc.allow_low_precision(reason="single bf16 rounding of 1/den"):
                nc.vector.reciprocal(out=rpk, in_=denpk)
            nc.sync.dma_start(out=rsc_d[b, :, :, :], in_=rpk)

            # ---- normalize U, add depthwise conv (PE diag matmuls), project ----
            for ct in range(CT):
                rbc = rbcp.tile([128, N], bf16, tag="rbc")
                nc.sync.dma_start(
                    out=rbc[0:64, :],
                    in_=rsc_d[b, 0:1, ct, :].to_broadcast((64, N)))
                nc.sync.dma_start(
                    out=rbc[64:128, :],
                    in_=rsc_d[b, 1:2, ct, :].to_broadcast((64, N)))
                for c in range(7):
                    psW = ps_mm.tile([128, 512], fp32, tag="mm")
                    for j in range(9):
                        dy, dx = j // 3, j % 3
                        nc.tensor.matmul(
                            psW[:, 0:C7],
                            wdiag_s[:, ct * 9 + j, :],
                            vpad[:, ct, 8 * c + dy:8 * c + dy + 8, dx:dx + 56],
                            start=(j == 0), stop=(j == 8),
                        )
                    sl = slice(c * C7, (c + 1) * C7)
                    nc.vector.tensor_mul(out=u_s[:, ct, sl], in0=u_s[:, ct, sl], in1=rbc[:, sl])
                    nc.vector.tensor_add(out=u_s[:, ct, sl], in0=u_s[:, ct, sl], in1=psW[:, 0:C7])
            for ci, (t0, cs) in enumerate(CH):
                psP = ps_mm.tile([128, 512], fp32, tag="mm")
                for kt in range(CT):
                    nc.tensor.matmul(
                        psP[0:cs, :], u_s[:, kt, t0:t0 + cs], pw_s[:, kt, :],
                        start=(kt == 0), stop=(kt == 3),
                    )
                ot = work.tile([128, 512], fp32, tag="ot")
                nc.scalar.copy(out=ot[0:cs, :], in_=psP[0:cs, :])
                nc.sync.dma_start(out=out_d[b, t0:t0 + cs, :], in_=ot[0:cs, :])
    return nc


def _host_prep(q_w, q_b, kv_w, kv_b, proj_w, proj_b, dwc_w, dwc_b,
               an_bias, na_bias, ah_bias, aw_bias, ha_bias, wa_bias):
    heads, dh = 8, 64
    scale = dh ** -0.5
    q_w = np.asarray(q_w, np.float32); q_b = np.asarray(q_b, np.float32)
    kv_w = np.asarray(kv_w, np.float32); kv_b = np.asarray(kv_b, np.float32)
    proj_w = np.asarray(proj_w, np.float32); proj_b = np.asarray(proj_b, np.float32)
    dwc_w = np.asarray(dwc_w, np.float32); dwc_b = np.asarray(dwc_b, np.float32)

    Rh = _resize_matrix(7, H)
    Rw = _resize_matrix(7, W)
    an = np.asarray(an_bias, np.float32); na = np.asarray(na_bias, np.float32)
    pb1 = np.einsum('yi,haij,xj->hayx', Rh, an, Rw).reshape(heads, A, N)
    pb2 = (np.asarray(ah_bias, np.float32) + np.asarray(aw_bias, np.float32)).reshape(heads, A, N)
    bias1 = pb1 + pb2                                      # (h, a, n)
    ab1 = np.einsum('yi,haij,xj->hayx', Rh, na, Rw).reshape(heads, A, N)
    ab2 = (np.asarray(ha_bias, np.float32) + np.asarray(wa_bias, np.float32)).reshape(heads, N, A)
    bias2 = ab1.transpose(0, 2, 1) + ab2                   # (h, n, a)

    k_w = kv_w[:, :512]
    v_w = kv_w[:, 512:]
    v_b = kv_b[512:]
    dwc9 = dwc_w.reshape(512, 9)

    wqk = np.concatenate([q_w * scale, k_w], axis=1)       # (512, 1024)
    wqk_t = np.ascontiguousarray(wqk.reshape(4, 128, 1024).transpose(1, 0, 2)).astype(BF)
    wv_t = np.ascontiguousarray(v_w.reshape(4, 128, 512).transpose(1, 0, 2)).astype(BF)
    pw_t = np.ascontiguousarray(proj_w.reshape(4, 128, 512).transpose(1, 0, 2)).astype(BF)
    qsb_t = np.ascontiguousarray((q_b * scale).reshape(4, 128).T).astype(np.float32)
    wdiag_t = np.zeros((128, 36, 128), np.float32)
    for ct_ in range(4):
        for j_ in range(9):
            wdiag_t[np.arange(128), ct_ * 9 + j_, np.arange(128)] = dwc9[ct_ * 128 + np.arange(128), j_]
    wdiag_t = wdiag_t.astype(BF)

    # eb1 (128, 25, HP, 128): [p, ch, hp, 64e+a] = exp(bias1)[2hp+e, a, 128ch+p]
    e1 = np.exp(bias1)                                     # (h, a, n)
    e1p = np.ones((128, 25, HP, 128), np.float32)
    e1t = e1.transpose(2, 0, 1)                            # (n, h, a)
    for ci, (t0, cs) in enumerate(CH):
        blk = e1t[t0:t0 + cs]                              # (cs, h, a)
        for hp_ in range(HP):
            e1p[:cs, ci, hp_, 0:49] = blk[:, 2 * hp_, :]
            e1p[:cs, ci, hp_, 64:113] = blk[:, 2 * hp_ + 1, :]
    eb1_t = e1p.astype(BF)

    # eb2 (128, HP, 7, 448): [64e+a, hp, c, t'] = exp(bias2)[2hp+e, 448c+t', a]
    e2 = np.exp(bias2)                                     # (h, n, a)
    e2p = np.zeros((128, HP, 7, C7), np.float32)
    for hp_ in range(HP):
        for e in range(2):
            e2p[64 * e:64 * e + 49, hp_] = e2[2 * hp_ + e].reshape(7, C7, A).transpose(2, 0, 1)
    eb2_t = e2p.astype(BF)

    # onesbd (128, 2): col e = 1 on rows 64e..64e+48, else 0
    ones_t = np.zeros((128, 2), np.float32)
    ones_t[0:49, 0] = 1.0
    ones_t[64:113, 1] = 1.0
    ones_t = ones_t.astype(BF)

    # host additive correction (v_b + dwc_b + proj_b, exact via softmax-sum-1)
    Mv = np.zeros((9, H, W), np.float32)
    for j in range(9):
        dy, dx = j // 3 - 1, j % 3 - 1
        Mv[j, max(0, -dy):H - max(0, dy), max(0, -dx):W - max(0, dx)] = 1.0
    S = np.einsum('jt,cj->tc', Mv.reshape(9, N), dwc9)
    corr = v_b[None, :] * (1.0 + S) + dwc_b[None, :]
    corr_out = (corr @ proj_w + proj_b[None, :]).astype(np.float32)   # (n, 512)

    return dict(wqk=wqk_t, wv=wv_t, pw=pw_t, qsb=qsb_t, wdiag=wdiag_t,
                eb1=eb1_t, eb2=eb2_t, onesbd=ones_t), corr_out


def kernel(**inputs):
    from concourse.bass_utils import run_bass_kernel_spmd

    x = np.asarray(inputs['x'], np.float32)                # (16, 3136, 512)
    shared, corr_out = _host_prep(
        inputs['q_w'], inputs['q_b'], inputs['kv_w'], inputs['kv_b'],
        inputs['proj_w'], inputs['proj_b'], inputs['dwc_w'], inputs['dwc_b'],
        inputs['an_bias'], inputs['na_bias'], inputs['ah_bias'],
        inputs['aw_bias'], inputs['ha_bias'], inputs['wa_bias'])

    # xT per core: (128, B, CT, N) bf16 ; [p, b, kt, t] = x[2c+b, t, 128kt+p]
    xb = np.ascontiguousarray(
        x.reshape(NCORES, B, N, CT, 128).transpose(0, 4, 1, 3, 2)).astype(BF)

    if 'nc' not in _CACHE:
        nc = _build_nc()
        nc.finalize()
        _CACHE['nc'] = nc
    nc = _CACHE['nc']

    in_maps = []
    for c in range(NCORES):
        m = {'xT': xb[c]}
        m.update(shared)
        in_maps.append(m)
    res = run_bass_kernel_spmd(nc, in_maps, core_ids=list(range(NCORES)))
    outs = res.results
    full = np.concatenate([np.asarray(o['out']).reshape(B, N, 512) for o in outs], axis=0)
    full = full + corr_out[None, :, :]
    return full.astype(np.float32)


# revision 22
# speedup vs baseline: 3.1780x; 1.3327x over previous
"""AgentAttention Trainium2 kernel — 8-core batch-parallel (2 batches/core).

Decomposition (validated in mirror.py against the reference):
  - host: x transposed to ch-major; q_w pre-scaled by dh^-0.5; k_b dropped
    (softmax-shift-invariant); v_b / dwc_b / proj_b folded into a host-side
    additive correction (softmax rows sum to 1); position biases precomputed
    as exp(bias) factors (multiplicative after exp, avoids PSUM bias adds).
  - device (per batch): Q^T projection ch-major; agent pooling via two
    strided DVE reduces; fused chunk-outer stage 1: K^T/V projected
    just-in-time per 128-token chunk, s1^T in n-partition layout with
    ones-augmented V for the softmax denominator; stage 2 in s2^T
    (agent-partition) layout with block-diagonal head-pair operands;
    denominators via ones-matmul; normalization as per-chunk broadcast
    multiply; depthwise 3x3 conv as 9 fused scalar_tensor_tensor taps on
    ch-major V (edge-cropped regions), split across Vector/GpSimd; final
    projection back to token-major.
"""
import numpy as np
import ml_dtypes

BF = ml_dtypes.bfloat16
NCORES = 8
B = 2              # batches per core
N = 3136
H = W = 56
CT = 4             # 128-channel tiles
HP = 4             # head pairs
A = 49
C7 = 448           # 8 image rows
CH = [(i * 128, min(128, N - i * 128)) for i in range(25)]

_CACHE = {}
DEBUG_TAPS = False


def _lin_weights(in_size, out_size):
    scale = in_size / out_size
    src = (np.arange(out_size, dtype=np.float32) + 0.5) * scale - 0.5
    src = np.maximum(src, 0.0)
    i0 = np.minimum(np.floor(src).astype(np.int32), in_size - 1)
    i1 = np.minimum(i0 + 1, in_size - 1)
    w = (src - i0.astype(np.float32)).astype(np.float32)
    return i0, i1, w


def _resize_matrix(in_size, out_size):
    i0, i1, w = _lin_weights(in_size, out_size)
    M = np.zeros((out_size, in_size), np.float32)
    M[np.arange(out_size), i0] += 1.0 - w
    M[np.arange(out_size), i1] += w
    return M


def _build_nc():
    from contextlib import ExitStack
    import concourse.bacc as bacc
    import concourse.tile as tile
    from concourse import mybir

    fp32 = mybir.dt.float32
    bf16 = mybir.dt.bfloat16
    AF = mybir.ActivationFunctionType
    OP = mybir.AluOpType
    AX = mybir.AxisListType

    nc = bacc.Bacc("TRN2", target_bir_lowering=False)
    xT_d = nc.dram_tensor("xT", (128, B, CT, N), bf16, kind="ExternalInput")
    wqk_d = nc.dram_tensor("wqk", (128, CT, 1024), bf16, kind="ExternalInput")
    wv_d = nc.dram_tensor("wv", (128, CT, 512), bf16, kind="ExternalInput")
    pw_d = nc.dram_tensor("pw", (128, CT, 512), bf16, kind="ExternalInput")
    qsb_d = nc.dram_tensor("qsb", (128, CT), fp32, kind="ExternalInput")
    wdiag_d = nc.dram_tensor("wdiag", (128, 36, 128), bf16, kind="ExternalInput")
    eb1_d = nc.dram_tensor("eb1", (128, 25, HP, 128), bf16, kind="ExternalInput")
    eb2_d = nc.dram_tensor("eb2", (128, HP, 7, C7), bf16, kind="ExternalInput")
    ones_d = nc.dram_tensor("onesbd", (128, 2), bf16, kind="ExternalInput")
    out_d = nc.dram_tensor("out", (B, N, 512), fp32, kind="ExternalOutput")
    rsc_d = nc.dram_tensor("rscratch", (B, 2, HP, N), bf16, kind="Internal")

    with ExitStack() as ctx:
        tc = ctx.enter_context(tile.TileContext(nc))
        consts = ctx.enter_context(tc.tile_pool(name="consts", bufs=1))
        xu = ctx.enter_context(tc.tile_pool(name="xu", bufs=2))
        qp = ctx.enter_context(tc.tile_pool(name="qp", bufs=1))
        kv = ctx.enter_context(tc.tile_pool(name="kv", bufs=2))
        biasp = ctx.enter_context(tc.tile_pool(name="biasp", bufs=3))
        work = ctx.enter_context(tc.tile_pool(name="work", bufs=3))
        batch1 = ctx.enter_context(tc.tile_pool(name="batch1", bufs=1))
        perb = ctx.enter_context(tc.tile_pool(name="perb", bufs=3))
        rbcp = ctx.enter_context(tc.tile_pool(name="rbcp", bufs=2))
        ps_mm = ctx.enter_context(tc.tile_pool(name="psmm", bufs=3, space="PSUM"))
        ps_av = ctx.enter_context(tc.tile_pool(name="psav", bufs=4, space="PSUM"))
        ps_sm = ctx.enter_context(tc.tile_pool(name="pssm", bufs=1, space="PSUM"))

        wqk_s = consts.tile([128, CT, 1024], bf16)
        nc.sync.dma_start(out=wqk_s, in_=wqk_d[:, :, :])
        wv_s = consts.tile([128, CT, 512], bf16)
        nc.sync.dma_start(out=wv_s, in_=wv_d[:, :, :])
        pw_s = consts.tile([128, CT, 512], bf16)
        nc.sync.dma_start(out=pw_s, in_=pw_d[:, :, :])
        qsb_s = consts.tile([128, CT], fp32)
        nc.sync.dma_start(out=qsb_s, in_=qsb_d[:, :])
        wdiag_s = consts.tile([128, 36, 128], bf16)
        nc.sync.dma_start(out=wdiag_s, in_=wdiag_d[:, :, :])
        onesbd = consts.tile([128, 2], bf16)
        nc.sync.dma_start(out=onesbd, in_=ones_d[:, :])

        for b in range(B):
            xT = xu.tile([128, CT, N], bf16, tag="xu")
            nc.sync.dma_start(out=xT, in_=xT_d[:, b, :, :])
            qT = qp.tile([128, CT, N], bf16, tag="qT")

            # ---- Q^T (ch-major, + bias, pre-scaled) ----
            for ct in range(CT):
                for c in range(7):
                    ps = ps_mm.tile([128, 512], fp32, tag="mm")
                    for kt in range(CT):
                        nc.tensor.matmul(
                            ps[:, 0:C7],
                            wqk_s[:, kt, ct * 128:(ct + 1) * 128],
                            xT[:, kt, c * C7:(c + 1) * C7],
                            start=(kt == 0), stop=(kt == 3),
                        )
                    nc.scalar.activation(
                        out=qT[:, ct, c * C7:(c + 1) * C7], in_=ps[:, 0:C7],
                        func=AF.Identity, bias=qsb_s[:, ct:ct + 1], scale=1.0,
                    )

            # ---- K^T (ch-major, resident) ----
            kT = kv.tile([128, CT, N], bf16, tag="kT")
            for ct in range(CT):
                for c in range(7):
                    ps = ps_mm.tile([128, 512], fp32, tag="mm")
                    for kt in range(CT):
                        nc.tensor.matmul(
                            ps[:, 0:C7],
                            wqk_s[:, kt, 512 + ct * 128:512 + (ct + 1) * 128],
                            xT[:, kt, c * C7:(c + 1) * C7],
                            start=(kt == 0), stop=(kt == 3),
                        )
                    nc.scalar.copy(out=kT[:, ct, c * C7:(c + 1) * C7], in_=ps[:, 0:C7])

            # ---- agent pooling -> block-diag head-pair operands ----
            asT = batch1.tile([128, CT, A], fp32, tag="asT")
            bd1 = batch1.tile([128, CT, 128], bf16, tag="bd1")
            bd2 = batch1.tile([128, CT, 128], bf16, tag="bd2")
            nc.vector.memset(bd1, 0.0)
            nc.vector.memset(bd2, 0.0)
            for ct in range(CT):
                p1 = work.tile([128, 392], fp32, tag="pool1")
                nc.vector.reduce_sum(
                    out=p1.rearrange("p (y q) -> p y q", y=56),
                    in_=qT[:, ct, :].rearrange("p (y q r) -> p y q r", y=56, q=7),
                    axis=AX.X,
                )
                nc.vector.reduce_sum(
                    out=asT[:, ct, :].rearrange("p (a c) -> p a c", a=7),
                    in_=p1.rearrange("p (yq yr xq) -> p yq xq yr", yq=7, yr=8),
                    axis=AX.X,
                )
                nc.scalar.mul(out=bd1[0:64, ct, 0:49], in_=asT[0:64, ct, :], mul=1.0 / 64.0)
                nc.scalar.mul(out=bd1[64:128, ct, 64:113], in_=asT[64:128, ct, :], mul=1.0 / 64.0)
                nc.scalar.mul(out=bd2[0:64, ct, 0:49], in_=asT[0:64, ct, :], mul=1.0 / 8.0)
                nc.scalar.mul(out=bd2[64:128, ct, 64:113], in_=asT[64:128, ct, :], mul=1.0 / 8.0)

            # ---- fused stage 1 (chunk-outer): all 4 head pairs per chunk,
            # ---- V^T projection units interleaved to keep PE dense ----
            vpad = kv.tile([128, CT, 58, 58], bf16, tag="kT")
            nc.vector.memset(vpad, 0.0)
            vt_units = [(ct_, r_) for ct_ in range(CT) for r_ in range(7)]
            def emit_vt(ct, r):
                ps = ps_mm.tile([128, 512], fp32, tag="mm")
                for kt in range(CT):
                    nc.tensor.matmul(
                        ps[:, 0:C7],
                        wv_s[:, kt, ct * 128:(ct + 1) * 128],
                        xT[:, kt, r * C7:(r + 1) * C7],
                        start=(kt == 0), stop=(kt == 3),
                    )
                nc.scalar.copy(
                    out=vpad[:, ct, 1 + 8 * r:9 + 8 * r, 1:57],
                    in_=ps[:, 0:C7].rearrange("p (y x) -> p y x", y=8))
            avps = []
            for _hp in range(HP):
                avp = ps_av.tile([128, 130], fp32, tag="av")
                avps.append(avp)
            for ci, (t0, cs) in enumerate(CH):
                v65 = perb.tile([128, 8, 65], bf16, tag="v65")
                ps = ps_mm.tile([128, 512], fp32, tag="mm")
                for kt in range(CT):
                    nc.tensor.matmul(
                        ps[0:cs, :], xT[:, kt, t0:t0 + cs], wv_s[:, kt, :],
                        start=(kt == 0), stop=(kt == 3),
                    )
                nc.vector.tensor_copy(
                    out=v65[0:cs, :, 0:64],
                    in_=ps[0:cs, :].rearrange("p (h d) -> p h d", h=8),
                )
                nc.vector.memset(v65[0:cs, :, 64:65], 1.0)
                e1b = biasp.tile([128, HP, 128], bf16, tag="eb1")
                nc.sync.dma_start(out=e1b, in_=eb1_d[:, ci, :, :])
                ps1 = ps_mm.tile([128, 512], fp32, tag="mm")
                for hp in range(HP):
                    nc.tensor.matmul(
                        ps1[0:cs, 128 * hp:128 * hp + 128],
                        kT[:, hp, t0:t0 + cs], bd1[:, hp, :],
                        start=True, stop=True,
                    )
                et4 = work.tile([128, HP, 128], bf16, tag="e1")
                nc.scalar.activation(
                    out=et4[0:cs, :, :].rearrange("p h a -> p (h a)"),
                    in_=ps1[0:cs, 0:512], func=AF.Exp)
                nc.vector.tensor_mul(out=et4[0:cs, :, :], in0=et4[0:cs, :, :], in1=e1b[0:cs, :, :])
                for hp in range(HP):
                    nc.tensor.matmul(
                        avps[hp][:, :],
                        et4[0:cs, hp, :],
                        v65[0:cs, 2 * hp:2 * hp + 2, :],
                        start=(ci == 0), stop=(ci == 24),
                    )
                if ci < 24 and vt_units:
                    emit_vt(*vt_units.pop(0))
            # normalize agent_v into block-diag lhsT tiles
            avbds = []
            for hp in range(HP):
                avbd = batch1.tile([128, 128], bf16, tag=f"avbd{hp}")
                nc.vector.memset(avbd, 0.0)
                rr = work.tile([128, 1], fp32, tag="rr")
                for e in range(2):
                    nc.vector.reciprocal(out=rr[64 * e:64 * e + 49, :],
                                         in_=avps[hp][64 * e:64 * e + 49, 65 * e + 64:65 * e + 65])
                    nc.vector.tensor_scalar_mul(
                        out=avbd[64 * e:64 * e + 49, 64 * e:64 * e + 64],
                        in0=avps[hp][64 * e:64 * e + 49, 65 * e:65 * e + 64],
                        scalar1=rr[64 * e:64 * e + 49, :],
                    )
                avbds.append(avbd)

            for u_ in list(vt_units):
                emit_vt(*u_)
            vt_units.clear()

            # ---- stage 2 per head pair: s2^T, E2, U + densum ----
            u_s = xu.tile([128, CT, N], bf16, tag="xu")
            dwc_s = kv.tile([128, CT, N], bf16, tag="kT")
            denpk = batch1.tile([112, 224], fp32, tag="denpk")
            for hp in range(HP):
                for c in range(7):
                    e2b = biasp.tile([128, C7], bf16, tag="eb2")
                    nc.sync.dma_start(out=e2b, in_=eb2_d[:, hp, c, :])
                    ps2 = ps_mm.tile([128, 512], fp32, tag="mm")
                    nc.tensor.matmul(
                        ps2[0:128, 0:C7], bd2[:, hp, :], qT[:, hp, c * C7:(c + 1) * C7],
                        start=True, stop=True,
                    )
                    et2 = work.tile([128, C7], bf16, tag="e2")
                    nc.scalar.activation(out=et2, in_=ps2[0:128, 0:C7], func=AF.Exp)
                    nc.vector.tensor_mul(out=et2, in0=et2, in1=e2b)
                    psU = ps_mm.tile([128, 512], fp32, tag="mm")
                    nc.tensor.matmul(psU[:, 0:C7], avbds[hp], et2, start=True, stop=True)
                    psD = ps_sm.tile([2, C7], fp32, tag="sm")
                    nc.tensor.matmul(psD, onesbd, et2, start=True, stop=True)
                    nc.scalar.copy(out=u_s[:, hp, c * C7:(c + 1) * C7], in_=psU[:, 0:C7])
                    dtmp = work.tile([2, C7], fp32, tag="dtmp")
                    nc.scalar.copy(out=dtmp, in_=psD)
                    for e in range(2):
                        nc.gpsimd.dma_start(
                            out=denpk[e * 56 + hp * 14 + 2 * c:e * 56 + hp * 14 + 2 * c + 2, :],
                            in_=dtmp[e:e + 1, :])
                    psW = ps_av.tile([128, 512], fp32, tag="av")
                    for j in range(9):
                        dy, dx = j // 3, j % 3
                        nc.tensor.matmul(
                            psW[:, 0:C7],
                            wdiag_s[:, hp * 9 + j, :],
                            vpad[:, hp, 8 * c + dy:8 * c + dy + 8, dx:dx + 56],
                            start=(j == 0), stop=(j == 8),
                        )
                    nc.scalar.copy(out=dwc_s[:, hp, c * C7:(c + 1) * C7], in_=psW[:, 0:C7])
            rpk = batch1.tile([112, 224], bf16, tag="rpk")
            with nc.allow_low_precision(reason="single bf16 rounding of 1/den"):
                nc.vector.reciprocal(out=rpk, in_=denpk)
            nc.sync.dma_start(out=rsc_d[b, :, :, :], in_=rpk)

            # ---- normalize U and add buffered depthwise conv ----
            for ct in range(CT):
                rbc = rbcp.tile([128, N], bf16, tag="rbc")
                nc.sync.dma_start(
                    out=rbc[0:64, :],
                    in_=rsc_d[b, 0:1, ct, :].to_broadcast((64, N)))
                nc.sync.dma_start(
                    out=rbc[64:128, :],
                    in_=rsc_d[b, 1:2, ct, :].to_broadcast((64, N)))
                nc.vector.tensor_mul(out=u_s[:, ct, :], in0=u_s[:, ct, :], in1=rbc)
                nc.vector.tensor_add(out=u_s[:, ct, :], in0=u_s[:, ct, :], in1=dwc_s[:, ct, :])
            for ci, (t0, cs) in enumerate(CH):
                psP = ps_mm.tile([128, 512], fp32, tag="mm")
                for kt in range(CT):
                    nc.tensor.matmul(
                        psP[0:cs, :], u_s[:, kt, t0:t0 + cs], pw_s[:, kt, :],
                        start=(kt == 0), stop=(kt == 3),
                    )
                ot = work.tile([128, 512], fp32, tag="ot")
                nc.scalar.copy(out=ot[0:cs, :], in_=psP[0:cs, :])
                nc.sync.dma_start(out=out_d[b, t0:t0 + cs, :], in_=ot[0:cs, :])
    return nc


def _host_prep(q_w, q_b, kv_w, kv_b, proj_w, proj_b, dwc_w, dwc_b,
               an_bias, na_bias, ah_bias, aw_bias, ha_bias, wa_bias):
    heads, dh = 8, 64
    scale = dh ** -0.5
    q_w = np.asarray(q_w, np.float32); q_b = np.asarray(q_b, np.float32)
    kv_w = np.asarray(kv_w, np.float32); kv_b = np.asarray(kv_b, np.float32)
    proj_w = np.asarray(proj_w, np.float32); proj_b = np.asarray(proj_b, np.float32)
    dwc_w = np.asarray(dwc_w, np.float32); dwc_b = np.asarray(dwc_b, np.float32)

    Rh = _resize_matrix(7, H)
    Rw = _resize_matrix(7, W)
    an = np.asarray(an_bias, np.float32); na = np.asarray(na_bias, np.float32)
    pb1 = np.einsum('yi,haij,xj->hayx', Rh, an, Rw).reshape(heads, A, N)
    pb2 = (np.asarray(ah_bias, np.float32) + np.asarray(aw_bias, np.float32)).reshape(heads, A, N)
    bias1 = pb1 + pb2                                      # (h, a, n)
    ab1 = np.einsum('yi,haij,xj->hayx', Rh, na, Rw).reshape(heads, A, N)
    ab2 = (np.asarray(ha_bias, np.float32) + np.asarray(wa_bias, np.float32)).reshape(heads, N, A)
    bias2 = ab1.transpose(0, 2, 1) + ab2                   # (h, n, a)

    k_w = kv_w[:, :512]
    v_w = kv_w[:, 512:]
    v_b = kv_b[512:]
    dwc9 = dwc_w.reshape(512, 9)

    wqk = np.concatenate([q_w * scale, k_w], axis=1)       # (512, 1024)
    wqk_t = np.ascontiguousarray(wqk.reshape(4, 128, 1024).transpose(1, 0, 2)).astype(BF)
    wv_t = np.ascontiguousarray(v_w.reshape(4, 128, 512).transpose(1, 0, 2)).astype(BF)
    pw_t = np.ascontiguousarray(proj_w.reshape(4, 128, 512).transpose(1, 0, 2)).astype(BF)
    qsb_t = np.ascontiguousarray((q_b * scale).reshape(4, 128).T).astype(np.float32)
    wdiag_t = np.zeros((128, 36, 128), np.float32)
    for ct_ in range(4):
        for j_ in range(9):
            wdiag_t[np.arange(128), ct_ * 9 + j_, np.arange(128)] = dwc9[ct_ * 128 + np.arange(128), j_]
    wdiag_t = wdiag_t.astype(BF)

    # eb1 (128, 25, HP, 128): [p, ch, hp, 64e+a] = exp(bias1)[2hp+e, a, 128ch+p]
    e1 = np.exp(bias1)                                     # (h, a, n)
    e1p = np.ones((128, 25, HP, 128), np.float32)
    e1t = e1.transpose(2, 0, 1)                            # (n, h, a)
    for ci, (t0, cs) in enumerate(CH):
        blk = e1t[t0:t0 + cs]                              # (cs, h, a)
        for hp_ in range(HP):
            e1p[:cs, ci, hp_, 0:49] = blk[:, 2 * hp_, :]
            e1p[:cs, ci, hp_, 64:113] = blk[:, 2 * hp_ + 1, :]
    eb1_t = e1p.astype(BF)

    # eb2 (128, HP, 7, 448): [64e+a, hp, c, t'] = exp(bias2)[2hp+e, 448c+t', a]
    e2 = np.exp(bias2)                                     # (h, n, a)
    e2p = np.zeros((128, HP, 7, C7), np.float32)
    for hp_ in range(HP):
        for e in range(2):
            e2p[64 * e:64 * e + 49, hp_] = e2[2 * hp_ + e].reshape(7, C7, A).transpose(2, 0, 1)
    eb2_t = e2p.astype(BF)

    # onesbd (128, 2): col e = 1 on rows 64e..64e+48, else 0
    ones_t = np.zeros((128, 2), np.float32)
    ones_t[0:49, 0] = 1.0
    ones_t[64:113, 1] = 1.0
    ones_t = ones_t.astype(BF)

    # host additive correction (v_b + dwc_b + proj_b, exact via softmax-sum-1)
    Mv = np.zeros((9, H, W), np.float32)
    for j in range(9):
        dy, dx = j // 3 - 1, j % 3 - 1
        Mv[j, max(0, -dy):H - max(0, dy), max(0, -dx):W - max(0, dx)] = 1.0
    S = np.einsum('jt,cj->tc', Mv.reshape(9, N), dwc9)
    corr = v_b[None, :] * (1.0 + S) + dwc_b[None, :]
    corr_out = (corr @ proj_w + proj_b[None, :]).astype(np.float32)   # (n, 512)

    return dict(wqk=wqk_t, wv=wv_t, pw=pw_t, qsb=qsb_t, wdiag=wdiag_t,
                eb1=eb1_t, eb2=eb2_t, onesbd=ones_t), corr_out


def kernel(**inputs):
    from concourse.bass_utils import run_bass_kernel_spmd

    x = np.asarray(inputs['x'], np.float32)                # (16, 3136, 512)
    shared, corr_out = _host_prep(
        inputs['q_w'], inputs['q_b'], inputs['kv_w'], inputs['kv_b'],
        inputs['proj_w'], inputs['proj_b'], inputs['dwc_w'], inputs['dwc_b'],
        inputs['an_bias'], inputs['na_bias'], inputs['ah_bias'],
        inputs['aw_bias'], inputs['ha_bias'], inputs['wa_bias'])

    # xT per core: (128, B, CT, N) bf16 ; [p, b, kt, t] = x[2c+b, t, 128kt+p]
    xb = np.ascontiguousarray(
        x.reshape(NCORES, B, N, CT, 128).transpose(0, 4, 1, 3, 2)).astype(BF)

    if 'nc' not in _CACHE:
        nc = _build_nc()
        nc.finalize()
        _CACHE['nc'] = nc
    nc = _CACHE['nc']

    in_maps = []
    for c in range(NCORES):
        m = {'xT': xb[c]}
        m.update(shared)
        in_maps.append(m)
    res = run_bass_kernel_spmd(nc, in_maps, core_ids=list(range(NCORES)))
    outs = res.results
    full = np.concatenate([np.asarray(o['out']).reshape(B, N, 512) for o in outs], axis=0)
    full = full + corr_out[None, :, :]
    return full.astype(np.float32)
